# revision 27
# baseline (speedup 1.0000x reference)
"""Multi-head attention (B=2, N=2048, D=1024, H=16, hd=64) on 8 TRN2 NeuronCores.

Sharding: data-parallel over batch (2) x tensor-parallel over heads (4 groups
of 4 heads). Each core computes, for its (batch b, head group g), the partial
output  outT_c[e, i] = sum_{d in shard} Wo[e, d] * O[i, d]  over its 256
sharded head dims; the host sums the 4 head-group partials per batch, adds bo.

v7: bf16 operands, host-preswizzled DRAM layouts, and ONE FLAT 128-step
software pipeline over all (head, i-half) attention windows: step k emits
S^T (2 x K=64 matmuls) + EXP for stream position k and PV for position k-2,
so the exp stream crosses window boundaries without draining.  QKV/O
projection fills are chunked into <=4-matmul pieces placed just ahead of
their consumers.  PSUM: s0/s1 [128,1024] score double-buffer, v0/v1 [65,512]
PV+Z accumulators (Z rides in PV row 64 via a memset ones column), p0/p1
projection scratch.  Every 1/Z normalize broadcasts via a K=1 PE matmul into
p-bank scratch (NO DRAM roundtrip), and output stores are batched into three
large DMAs — the NeuronCore sync sequencer costs ~1.8us per DMA event, so
the kernel issues only ~12 DMAs total.  The tail out-projection prefills
pair-0 + the h2 quarter into s/p banks before the last normalize chain (plus
zero-accumulate keep-warm matmuls to hold PE p-state), leaving only the h3
K=64 quarter after it; evacuations alternate between ACT and DVE.
"""
import sys

sys.path.insert(0, "/opt/trn_rl_repo")

import ml_dtypes
import numpy as np

import concourse.bass as bass
import concourse.tile as tile
from concourse import bacc, bass_utils, mybir

P = 128
NTOK = 2048          # sequence length
D = 1024             # model dim
HPC = 4              # heads per core
HD = 64              # head dim
DSH = HPC * HD       # 256: sharded head dims per core
CO = 8               # contraction chunks over c (D/P)
NIH = 2              # i halves
IHW = NTOK // NIH    # 1024
NJB = NTOK // P      # 16 j blocks
SCALE = HD ** -0.5

F32 = mybir.dt.float32
BF16 = mybir.dt.bfloat16
NP_BF16 = ml_dtypes.bfloat16
EXP_FN = mybir.ActivationFunctionType.Exp

# window order: all of ih0 (h0..h3), then ih1 (h0..h3)
WINS = [(0, 0), (1, 0), (2, 0), (3, 0), (0, 1), (1, 1), (2, 1), (3, 1)]
NW = len(WINS)


def build_nc():
    nc = bacc.Bacc("TRN2", target_bir_lowering=False, debug=False)

    # host-preswizzled inputs: each DMA is contiguous per partition
    xq_d = [None, None, None, None]
    x0a_d = nc.dram_tensor("xq0a", [P, 4, 512], BF16, kind="ExternalInput").ap()
    x0b_d = nc.dram_tensor("xq0b", [P, 4, 512], BF16, kind="ExternalInput").ap()
    for q in (1, 2, 3):
        xq_d[q] = nc.dram_tensor(f"xq{q}", [P, CO, 512], BF16,
                                 kind="ExternalInput").ap()
    wq_d = nc.dram_tensor("wqs", [P, CO, DSH], BF16, kind="ExternalInput").ap()
    wk_d = nc.dram_tensor("wks", [P, CO, DSH], BF16, kind="ExternalInput").ap()
    wv_d = nc.dram_tensor("wvs", [P, CO, DSH], BF16, kind="ExternalInput").ap()
    wo_d = nc.dram_tensor("wos", [P, 2, D], BF16, kind="ExternalInput").ap()
    outt_d = nc.dram_tensor("outt", [D, NTOK], BF16, kind="ExternalOutput").ap()
    out_t = outt_d.rearrange("(m p) i -> p m i", p=P)     # [128, 8, 2048]

    with tile.TileContext(nc) as tc:
        with (
            tc.tile_pool(name="sbp", bufs=1) as sbp,           # persistent
            tc.tile_pool(name="sbw", bufs=1) as sbw,           # working
            tc.tile_pool(name="ps", bufs=1, space="PSUM") as ps,
            tc.tile_pool(name="dr", bufs=2, space="DRAM") as dr,
        ):
            # ---------------- persistent tiles ----------------
            qt = sbp.tile([P, 2, NTOK], BF16, tag="qt")        # Q^T natural
            ktp = sbp.tile([P, HPC, NTOK], BF16, tag="ktp")    # K^T half-rows
            vaug = sbp.tile([P, NJB, HPC, 65], BF16, tag="vaug")  # V | ones
            ota = sbp.tile([P, 2, NTOK], BF16, tag="ota")      # O^T all heads
            wo = sbp.tile([P, 2, D], BF16, tag="wo")
            wq = sbp.tile([P, CO, DSH], BF16, tag="wq")
            wk = sbp.tile([P, CO, DSH], BF16, tag="wk")
            wv = sbp.tile([P, CO, DSH], BF16, tag="wv")
            x0a = sbp.tile([P, 4, 512], BF16, tag="x0a")
            x0b = sbp.tile([P, 4, 512], BF16, tag="x0b")
            xq = [None] + [sbp.tile([P, CO, 512], BF16, tag=f"xq{q}",
                                    name=f"xq{q}") for q in (1, 2, 3)]
            onesb = sbp.tile([1, HD], BF16, tag="onesb")       # bcast lhsT
            warm = sbp.tile([P, 512], BF16, tag="warm")        # p-state fuel
            zerot = sbp.tile([P, P], BF16, tag="zerot")        # 0-accumulate
            stg0 = sbp.tile([P, 8, IHW], BF16, tag="stg0")     # out ih0 stage
            stg1 = sbp.tile([P, 8, IHW], BF16, tag="stg1")     # out ih1 stage

            def xsl(q, o, lo=0, hi=512):
                if q == 0:
                    t = x0a if o < 4 else x0b
                    return t[:, o % 4, lo:hi]
                return xq[q][:, o, lo:hi]

            # ---------------- DMA issue (arrival order matters) -------------
            nc.sync.dma_start(wk[:], wk_d)
            nc.sync.dma_start(x0a[:], x0a_d)
            nc.sync.dma_start(wq[:], wq_d)
            nc.sync.dma_start(x0b[:], x0b_d)
            nc.sync.dma_start(xq[1][:], xq_d[1])
            nc.sync.dma_start(wv[:], wv_d)
            nc.sync.dma_start(xq[2][:], xq_d[2])
            nc.sync.dma_start(xq[3][:], xq_d[3])
            nc.sync.dma_start(wo[:], wo_d)
            nc.vector.memset(vaug[:, :, :, 64:65], 1.0)
            nc.vector.memset(onesb[:], 1.0)
            nc.vector.memset(warm[:], 0.25)
            nc.vector.memset(zerot[:], 0.0)

            # ---------------- filler builders ----------------
            pcycle = [0]

            def ptag():
                pcycle[0] += 1
                return f"p{pcycle[0] % 2}"

            def kq_chunks(is_q, mt, ihh, c):
                """Q^T/K^T [128,512] fill split into two 4-matmul chunks."""
                cell = {}
                w_sb = wq if is_q else wk

                def emit(o0, o1):
                    for o in range(o0, o1):
                        nc.tensor.matmul(
                            cell["pp"][:],
                            w_sb[:, o, mt * P:(mt + 1) * P],
                            xsl(2 * ihh + c, o),
                            start=(o == 0), stop=(o == CO - 1),
                        )

                def a():
                    t = ptag()
                    cell["pp"] = ps.tile([P, 512], F32, tag=t, name=f"ps_{t}")
                    emit(0, 4)

                def b():
                    emit(4, CO)
                    pp = cell["pp"]
                    sl = slice(ihh * IHW + c * 512, ihh * IHW + (c + 1) * 512)
                    if is_q:
                        nc.vector.tensor_copy(qt[:, mt, sl], pp[:])
                    else:
                        nc.vector.tensor_copy(ktp[0:64, 2 * mt, sl],
                                              pp[0:64, :])
                        nc.vector.tensor_copy(ktp[64:128, 2 * mt + 1, sl],
                                              pp[64:128, :])
                return a, b

            def kq_fill(is_q, mt, ihh, c):
                a, b = kq_chunks(is_q, mt, ihh, c)
                a()
                b()

            def v_fill(it, pair):
                """V projection for token block it, head pair `pair`."""
                t = ptag()
                pp = ps.tile([P, P], F32, tag=t, name=f"ps_{t}")
                ihh, loc = divmod(it, 8)
                q, lb = 2 * ihh + loc // 4, loc % 4
                for o in range(CO):
                    nc.tensor.matmul(
                        pp[:],
                        xsl(q, o, lb * P, (lb + 1) * P),
                        wv[:, o, pair * P:(pair + 1) * P],
                        start=(o == 0), stop=(o == CO - 1),
                    )
                nc.vector.tensor_copy(
                    vaug[:, it, 2 * pair:2 * pair + 2, 0:64],
                    pp[:].rearrange("p (h d) -> p h d", d=HD),
                )

            def oproj_chunks(mt, ihh):
                """ih0 output-projection row-tile as two per-c chunks,
                staged into stg0 (stored once at the end of the stream)."""
                def piece(c):
                    t = ptag()
                    pp = ps.tile([P, 512], F32, tag=t, name=f"ps_{t}")
                    for o in range(2):
                        nc.tensor.matmul(
                            pp[:],
                            wo[:, o, mt * P:(mt + 1) * P],
                            ota[:, o, ihh * IHW + c * 512:
                                ihh * IHW + (c + 1) * 512],
                            start=(o == 0), stop=(o == 1),
                        )
                    nc.vector.tensor_copy(
                        stg0[:, mt, c * 512:(c + 1) * 512], pp[:])
                return (lambda: piece(0)), (lambda: piece(1))

            # ---------------- normalize ----------------
            def norm_window(w, vc, fast):
                """1/Z normalize for window w.  In-stream windows broadcast
                1/Z through a DRAM roundtrip and return a deferred final-
                multiply closure (runs ~6 steps later, once the broadcast has
                landed); the last window broadcasts via a K=1 PE matmul."""
                h, ihh = WINS[w]
                row = slice((h % 2) * 64, (h % 2) * 64 + 64)
                isl = slice(ihh * IHW, (ihh + 1) * IHW)
                ot = sbw.tile([64, IHW], F32, tag="ot", bufs=2, name="ot")
                zt = sbw.tile([1, IHW], F32, tag="zt", bufs=2, name="zt")
                rt = sbw.tile([1, IHW], F32, tag="rt", bufs=2, name="rt")
                for c in range(2):
                    cs = slice(c * 512, (c + 1) * 512)
                    nc.vector.tensor_copy(zt[:, cs], vc[c][64:65, :])
                    if fast:   # ACT is idle after the last EXP
                        nc.scalar.copy(ot[:, cs], vc[c][0:64, :])
                    else:
                        nc.vector.tensor_copy(ot[:, cs], vc[c][0:64, :])
                nc.vector.reciprocal_approx_fast(out=rt[:], in_=zt[:])
                if fast:
                    rtb = sbw.tile([1, IHW], BF16, tag="rtb", name="rtb")
                    nc.vector.tensor_copy(rtb[:], rt[:])
                    for c in range(2):
                        cs = slice(c * 512, (c + 1) * 512)
                        rbp = ps.tile([HD, 512], F32, tag=f"v{c}",
                                      name=f"ps_rb{c}")
                        nc.tensor.matmul(rbp[:], onesb[:], rtb[:, cs],
                                         start=True, stop=True)
                        nc.vector.tensor_mul(
                            ota[row, h // 2, ihh * IHW + c * 512:
                                ihh * IHW + (c + 1) * 512],
                            ot[:, cs], rbp[:])
                    return None
                rdram = dr.tile([1, IHW], F32, tag="rd")
                nc.sync.dma_start(rdram[:], rt[:])
                rb = sbw.tile([64, IHW], F32, tag="rb", bufs=2, name="rb")
                nc.sync.dma_start(rb[:], rdram[:].to_broadcast((64, IHW)))

                def finish():
                    nc.vector.tensor_mul(ota[row, h // 2, isl], ot[:], rb[:])
                return finish

            # ---------------- tail out-projection pieces ----------------
            st_parts = {}
            FULL, H2, H3 = slice(0, P), slice(0, 64), slice(64, P)

            def tail_mm(dst, o_rows, o, mt, c, start, stop):
                nc.tensor.matmul(
                    dst,
                    wo[o_rows, o, mt * P:(mt + 1) * P],
                    ota[o_rows, o, IHW + c * 512: IHW + (c + 1) * 512],
                    start=start, stop=stop,
                )

            def tail_prefill():
                for mt in range(3):
                    if mt < 2:
                        st = ps.tile([P, IHW], F32, tag=f"s{mt % 2}",
                                     name=f"ps_st{mt % 2}")
                        parts = (st,)
                    else:
                        parts = tuple(
                            ps.tile([P, 512], F32, tag=f"p{i}", name=f"ps_p{i}")
                            for i in range(2))
                    st_parts[mt] = parts
                    for c in range(2):
                        dst = (parts[c][:] if len(parts) == 2
                               else parts[0][:, c * 512:(c + 1) * 512])
                        tail_mm(dst, FULL, 0, mt, c, True, False)
                        tail_mm(dst, H2, 1, mt, c, False, False)
                # zero-accumulate keep-warm matmuls: hold PE p-state through
                # the normalize chain without changing the open groups.
                for i in range(18):
                    st0 = st_parts[0][0]
                    nc.tensor.matmul(st0[:, 0:512], zerot[:], warm[:],
                                     start=False, stop=False)

            # ---------------- the flat pipeline ----------------
            # warm-up: keep the PE busy through the input DMA so it reaches
            # full p-state before the lead-in fills (which are DMA-paced).
            for i in range(12):
                wps = ps.tile([64, 512], F32, tag=f"v{i % 2}",
                              name=f"ps_wm{i % 2}")
                nc.tensor.matmul(wps[:], warm[:, 0:64], warm[:],
                                 start=True, stop=True)

            # lead-in: K^T heads 0,1 j 0:512; Q^T heads 0,1 ih0 c0.
            # (Q0 c1 is emitted inside step 0, after the first half-EXP, so
            # the exp stream starts as soon as K0f0+Q0f0 land.)
            kq_fill(False, 0, 0, 0)
            kq_fill(True, 0, 0, 0)

            from collections import defaultdict
            pre = defaultdict(list)
            mid = defaultdict(list)

            def place(d, step, *fns):
                for i, f in enumerate(fns):
                    d[step + i].append(f)

            # w0: K0 quarters chunked+paced; V pair0 paced per jb
            place(pre, 1, *kq_chunks(False, 0, 0, 1))
            place(pre, 5, *kq_chunks(False, 0, 1, 0))
            place(pre, 9, *kq_chunks(False, 0, 1, 1))
            for it in range(NJB):
                place(mid, it, lambda it=it: v_fill(it, 0))
            # w1: Q1 ih0; first K1 quarter
            place(pre, 16, *kq_chunks(True, 1, 0, 0))
            place(pre, 18, *kq_chunks(True, 1, 0, 1))
            place(pre, 28, *kq_chunks(False, 1, 0, 0))
            # w2: V pair1 paced one step ahead of its PV; K1 quarters
            place(pre, 32, *kq_chunks(False, 1, 0, 1))
            place(pre, 36, *kq_chunks(False, 1, 1, 0))
            place(pre, 40, *kq_chunks(False, 1, 1, 1))
            for it in range(NJB):
                place(mid, 32 + it, lambda it=it: v_fill(it, 1))
            # w3: Q0 ih1 (due w4)
            place(pre, 48, *kq_chunks(True, 0, 1, 0))
            place(pre, 52, *kq_chunks(True, 0, 1, 1))
            # w4: Q1 ih1 (due w6); out-proj(ih0) row-tiles 0,1
            place(pre, 64, *kq_chunks(True, 1, 1, 0))
            place(pre, 68, *kq_chunks(True, 1, 1, 1))
            place(pre, 72, *oproj_chunks(0, 0))
            place(pre, 76, *oproj_chunks(1, 0))
            # w5: out-proj(ih0) row-tiles 2..5
            for i, mt in enumerate((2, 3, 4, 5)):
                place(pre, 80 + 4 * i, *oproj_chunks(mt, 0))
            # w6: out-proj(ih0) row-tiles 6,7; single batched store after
            place(pre, 96, *oproj_chunks(6, 0))
            place(pre, 100, *oproj_chunks(7, 0))
            place(pre, 103,
                  lambda: nc.sync.dma_start(out_t[:, :, 0:IHW], stg0[:]))

            es_q = {}
            vc = None
            NSTEP = NW * NJB
            for k in range(NSTEP + 2):
                for f in pre.get(k, ()):
                    f()
                if k < NSTEP:
                    w, jb = divmod(k, NJB)
                    h, ihh = WINS[w]
                    row = slice((h % 2) * 64, (h % 2) * 64 + 64)
                    ss = ps.tile([P, IHW], F32, tag=f"s{k % 2}",
                                 name=f"ps_s{k % 2}")
                    es = sbw.tile([P, IHW], BF16, tag="es", bufs=4, name="es")

                    def s_mm(c):
                        nc.tensor.matmul(
                            ss[:, c * 512:(c + 1) * 512],
                            ktp[row, h, jb * P:(jb + 1) * P],
                            qt[row, h // 2, ihh * IHW + c * 512:
                               ihh * IHW + (c + 1) * 512],
                            start=True, stop=True,
                        )
                    if k == 0:
                        # half-width exps: start the stream on Q0f0 alone
                        s_mm(0)
                        nc.scalar.activation(es[:, 0:512], ss[:, 0:512],
                                             EXP_FN)
                        kq_fill(True, 0, 0, 1)
                        s_mm(1)
                        nc.scalar.activation(es[:, 512:1024],
                                             ss[:, 512:1024], EXP_FN)
                    else:
                        s_mm(0)
                        s_mm(1)
                        nc.scalar.activation(es[:], ss[:], EXP_FN)
                    es_q[k] = es
                for f in mid.get(k, ()):
                    f()
                p = k - 2
                if p < 0:
                    continue
                w, pj = divmod(p, NJB)
                h, ihh = WINS[w]
                if pj == 0:
                    vc = [ps.tile([65, 512], F32, tag=f"v{c}", name=f"ps_v{c}")
                          for c in range(2)]
                pes = es_q.pop(p)
                for c in range(2):
                    nc.tensor.matmul(
                        vc[c][:],
                        vaug[:, pj, h, 0:65],
                        pes[:, c * 512:(c + 1) * 512],
                        start=(pj == 0), stop=(pj == NJB - 1),
                    )
                if pj == NJB - 1:
                    last = (w == NW - 1)
                    if last:
                        tail_prefill()
                    fin = norm_window(w, vc, fast=last)
                    if fin is not None:
                        place(pre, k + 6, fin)

            # tail: finish the ih1 out-projection; evacuation alternates
            # between the idle ACT and DVE engines.  Prefilled row-tiles
            # must finish first — they hold the s/p banks open.
            for mt in range(8):
                if mt in st_parts:
                    parts = st_parts[mt]
                else:
                    parts = (ps.tile([P, IHW], F32, tag=f"s{mt % 2}",
                                     name=f"ps_st{mt % 2}"),)
                for c in range(2):
                    cs = slice(c * 512, (c + 1) * 512)
                    dst = parts[c][:] if len(parts) == 2 else parts[0][:, cs]
                    if mt in st_parts:
                        tail_mm(dst, H3, 1, mt, c, False, True)
                    else:
                        tail_mm(dst, FULL, 0, mt, c, True, False)
                        tail_mm(dst, FULL, 1, mt, c, False, True)
                    if (2 * mt + c) % 2:
                        nc.scalar.copy(stg1[:, mt, cs], dst)
                    else:
                        nc.vector.tensor_copy(stg1[:, mt, cs], dst)
                nc.sync.dma_start(out_t[:, mt, IHW:NTOK], stg1[:, mt, :])

    nc.compile()
    return nc


_NC_CACHE = None


def _get_nc():
    global _NC_CACHE
    if _NC_CACHE is None:
        _NC_CACHE = build_nc()
    return _NC_CACHE


def _swz(a, po):
    """[po*128, rest] -> [128, po, rest] host swizzle (contiguous/partition)."""
    rest = a.shape[1]
    return np.ascontiguousarray(
        a.reshape(po, P, rest).transpose(1, 0, 2)).astype(NP_BF16)


def kernel(x, Wq, Wk, Wv, Wo, bo, _trace=False):
    x = np.asarray(x, dtype=np.float32)
    Wq = np.asarray(Wq, dtype=np.float32)
    Wk = np.asarray(Wk, dtype=np.float32)
    Wv = np.asarray(Wv, dtype=np.float32)
    Wo = np.asarray(Wo, dtype=np.float32)
    bo = np.asarray(bo, dtype=np.float32)
    B = x.shape[0]

    nc = _get_nc()
    in_maps = []
    for core in range(8):
        b, hg = divmod(core, 4)
        rows = slice(hg * DSH, (hg + 1) * DSH)
        xs = _swz(np.ascontiguousarray(x[b].T), CO)      # [128, 8, 2048]
        m = {"xq0a": np.ascontiguousarray(xs[:, 0:4, 0:512]),
             "xq0b": np.ascontiguousarray(xs[:, 4:8, 0:512])}
        for q in (1, 2, 3):
            m[f"xq{q}"] = np.ascontiguousarray(xs[:, :, q * 512:(q + 1) * 512])
        m["wqs"] = _swz(np.ascontiguousarray(Wq[rows, :].T), CO)
        m["wks"] = _swz(np.ascontiguousarray((Wk[rows, :] * SCALE).T), CO)
        m["wvs"] = _swz(np.ascontiguousarray(Wv[rows, :].T), CO)
        m["wos"] = _swz(np.ascontiguousarray(Wo[:, rows].T), 2)
        in_maps.append(m)

    res = bass_utils.run_bass_kernel_spmd(
        nc, in_maps, core_ids=list(range(8)), trace=_trace)

    out = np.zeros((B, NTOK, D), dtype=np.float32)
    for core in range(8):
        b = core // 4
        out[b] += res.results[core]["outt"].astype(np.float32).T
    out += bo
    if _trace:
        kernel.last_results = res
    return out


# revision 30
# speedup vs baseline: 1.1926x; 1.1926x over previous
"""Multi-head attention (B=2, N=2048, D=1024, H=16, hd=64) on 8 TRN2 NeuronCores.

Sharding: data-parallel over batch (2) x tensor-parallel over heads (4 groups
of 4 heads). Each core computes, for its (batch b, head group g), the partial
output  outT_c[e, i] = sum_{d in shard} Wo[e, d] * O[i, d]  over its 256
sharded head dims; the host sums the 4 head-group partials per batch, adds bo.

v7: bf16 operands, host-preswizzled DRAM layouts, and ONE FLAT 128-step
software pipeline over all (head, i-half) attention windows: step k emits
S^T (2 x K=64 matmuls) + EXP for stream position k and PV for position k-2,
so the exp stream crosses window boundaries without draining.  QKV/O
projection fills are chunked into <=4-matmul pieces placed just ahead of
their consumers.  PSUM: s0/s1 [128,1024] score double-buffer, v0/v1 [65,512]
PV+Z accumulators (Z rides in PV row 64 via a memset ones column), p0/p1
projection scratch.  Every 1/Z normalize broadcasts via a K=1 PE matmul into
p-bank scratch (NO DRAM roundtrip), and output stores are batched into three
large DMAs — the NeuronCore sync sequencer costs ~1.8us per DMA event, so
the kernel issues only ~12 DMAs total.  The tail out-projection prefills
pair-0 + the h2 quarter into s/p banks before the last normalize chain (plus
zero-accumulate keep-warm matmuls to hold PE p-state), leaving only the h3
K=64 quarter after it; evacuations alternate between ACT and DVE.
"""
import sys

sys.path.insert(0, "/opt/trn_rl_repo")

import ml_dtypes
import numpy as np

import concourse.bass as bass
import concourse.tile as tile
from concourse import bacc, bass_utils, mybir

P = 128
NTOK = 2048          # sequence length
D = 1024             # model dim
HPC = 4              # heads per core
HD = 64              # head dim
DSH = HPC * HD       # 256: sharded head dims per core
CO = 8               # contraction chunks over c (D/P)
NIH = 2              # i halves
IHW = NTOK // NIH    # 1024
NJB = NTOK // P      # 16 j blocks
SCALE = HD ** -0.5

F32 = mybir.dt.float32
BF16 = mybir.dt.bfloat16
NP_BF16 = ml_dtypes.bfloat16
EXP_FN = mybir.ActivationFunctionType.Exp

# window order: all of ih0 (h0..h3), then ih1 (h0..h3)
WINS = [(0, 0), (1, 0), (2, 0), (3, 0), (0, 1), (1, 1), (2, 1), (3, 1)]
NW = len(WINS)


def build_nc():
    nc = bacc.Bacc("TRN2", target_bir_lowering=False, debug=False)

    # host-preswizzled inputs: each DMA is contiguous per partition
    xq_d = [None, None, None, None]
    x0a_d = nc.dram_tensor("xq0a", [P, 4, 512], BF16, kind="ExternalInput").ap()
    x0b_d = nc.dram_tensor("xq0b", [P, 4, 512], BF16, kind="ExternalInput").ap()
    for q in (1, 2, 3):
        xq_d[q] = nc.dram_tensor(f"xq{q}", [P, CO, 512], BF16,
                                 kind="ExternalInput").ap()
    wq_d = nc.dram_tensor("wqs", [P, CO, DSH], BF16, kind="ExternalInput").ap()
    wk_d = nc.dram_tensor("wks", [P, CO, DSH], BF16, kind="ExternalInput").ap()
    wv_d = nc.dram_tensor("wvs", [P, CO, DSH], BF16, kind="ExternalInput").ap()
    wo_d = nc.dram_tensor("wos", [P, 2, D], BF16, kind="ExternalInput").ap()
    outt_d = nc.dram_tensor("outt", [D, NTOK], BF16, kind="ExternalOutput").ap()
    out_t = outt_d.rearrange("(m p) i -> p m i", p=P)     # [128, 8, 2048]

    with tile.TileContext(nc) as tc:
        with (
            tc.tile_pool(name="sbp", bufs=1) as sbp,           # persistent
            tc.tile_pool(name="sbw", bufs=1) as sbw,           # working
            tc.tile_pool(name="ps", bufs=1, space="PSUM") as ps,
            tc.tile_pool(name="dr", bufs=2, space="DRAM") as dr,
        ):
            # ---------------- persistent tiles ----------------
            qt = sbp.tile([P, 2, NTOK], BF16, tag="qt")        # Q^T natural
            ktp = sbp.tile([P, HPC, NTOK], BF16, tag="ktp")    # K^T half-rows
            vaug = sbp.tile([P, NJB, HPC, 65], BF16, tag="vaug")  # V | ones
            ota = sbp.tile([P, 2, NTOK], BF16, tag="ota")      # O^T all heads
            wo = sbp.tile([P, 2, D], BF16, tag="wo")
            wq = sbp.tile([P, CO, DSH], BF16, tag="wq")
            wk = sbp.tile([P, CO, DSH], BF16, tag="wk")
            wv = sbp.tile([P, CO, DSH], BF16, tag="wv")
            x0a = sbp.tile([P, 4, 512], BF16, tag="x0a")
            x0b = sbp.tile([P, 4, 512], BF16, tag="x0b")
            xq = [None] + [sbp.tile([P, CO, 512], BF16, tag=f"xq{q}",
                                    name=f"xq{q}") for q in (1, 2, 3)]
            onesb = sbp.tile([1, HD], BF16, tag="onesb")       # bcast lhsT
            warm = sbp.tile([P, 512], BF16, tag="warm")        # p-state fuel
            zerot = sbp.tile([P, P], BF16, tag="zerot")        # 0-accumulate
            stg0 = sbp.tile([P, 8, IHW], BF16, tag="stg0")     # out ih0 stage
            stg1 = sbp.tile([P, 8, IHW], BF16, tag="stg1")     # out ih1 stage

            def xsl(q, o, lo=0, hi=512):
                if q == 0:
                    t = x0a if o < 4 else x0b
                    return t[:, o % 4, lo:hi]
                return xq[q][:, o, lo:hi]

            # ---------------- DMA issue (arrival order matters) -------------
            nc.sync.dma_start(wk[:], wk_d)
            nc.sync.dma_start(x0a[:], x0a_d)
            nc.sync.dma_start(wq[:], wq_d)
            nc.sync.dma_start(x0b[:], x0b_d)
            nc.sync.dma_start(xq[1][:], xq_d[1])
            nc.sync.dma_start(wv[:], wv_d)
            nc.sync.dma_start(xq[2][:], xq_d[2])
            nc.sync.dma_start(xq[3][:], xq_d[3])
            nc.sync.dma_start(wo[:], wo_d)
            nc.vector.memset(vaug[:, :, :, 64:65], 1.0)
            nc.vector.memset(onesb[:], 1.0)
            nc.vector.memset(warm[:], 0.25)
            nc.vector.memset(zerot[:], 0.0)

            # ---------------- filler builders ----------------
            pcycle = [0]

            def ptag():
                pcycle[0] += 1
                return f"p{pcycle[0] % 2}"

            def kq_chunks(is_q, mt, ihh, c, nparts=2):
                """Q^T/K^T [128,512] fill split into `nparts` matmul chunks
                (each chunk fits a step's PE slack; the psum accumulation
                group stays open across the chunks)."""
                cell = {}
                w_sb = wq if is_q else wk
                per = CO // nparts

                def emit(o0, o1):
                    for o in range(o0, o1):
                        nc.tensor.matmul(
                            cell["pp"][:],
                            w_sb[:, o, mt * P:(mt + 1) * P],
                            xsl(2 * ihh + c, o),
                            start=(o == 0), stop=(o == CO - 1),
                        )

                def mk(i):
                    def f():
                        if i == 0:
                            t = ptag()
                            cell["pp"] = ps.tile([P, 512], F32, tag=t,
                                                 name=f"ps_{t}")
                        emit(i * per, (i + 1) * per)
                        if i == nparts - 1:
                            pp = cell["pp"]
                            sl = slice(ihh * IHW + c * 512,
                                       ihh * IHW + (c + 1) * 512)
                            if is_q:
                                nc.vector.tensor_copy(qt[:, mt, sl], pp[:])
                            else:
                                nc.vector.tensor_copy(
                                    ktp[0:64, 2 * mt, sl], pp[0:64, :])
                                nc.vector.tensor_copy(
                                    ktp[64:128, 2 * mt + 1, sl],
                                    pp[64:128, :])
                    return f
                return tuple(mk(i) for i in range(nparts))

            def kq_fill(is_q, mt, ihh, c):
                for f in kq_chunks(is_q, mt, ihh, c):
                    f()

            def v_fill(it, pair):
                """V projection for token block it, head pair `pair`."""
                t = ptag()
                pp = ps.tile([P, P], F32, tag=t, name=f"ps_{t}")
                ihh, loc = divmod(it, 8)
                q, lb = 2 * ihh + loc // 4, loc % 4
                for o in range(CO):
                    nc.tensor.matmul(
                        pp[:],
                        xsl(q, o, lb * P, (lb + 1) * P),
                        wv[:, o, pair * P:(pair + 1) * P],
                        start=(o == 0), stop=(o == CO - 1),
                    )
                nc.vector.tensor_copy(
                    vaug[:, it, 2 * pair:2 * pair + 2, 0:64],
                    pp[:].rearrange("p (h d) -> p h d", d=HD),
                )

            def oproj_chunks(mt, ihh):
                """ih0 output-projection row-tile as two per-c chunks,
                staged into stg0 (stored once at the end of the stream)."""
                def piece(c):
                    t = ptag()
                    pp = ps.tile([P, 512], F32, tag=t, name=f"ps_{t}")
                    for o in range(2):
                        nc.tensor.matmul(
                            pp[:],
                            wo[:, o, mt * P:(mt + 1) * P],
                            ota[:, o, ihh * IHW + c * 512:
                                ihh * IHW + (c + 1) * 512],
                            start=(o == 0), stop=(o == 1),
                        )
                    nc.vector.tensor_copy(
                        stg0[:, mt, c * 512:(c + 1) * 512], pp[:])
                return (lambda: piece(0)), (lambda: piece(1))

            # ---------------- normalize ----------------
            def norm_window(w, vc, fast):
                """1/Z normalize for window w.  In-stream windows broadcast
                1/Z through a DRAM roundtrip and return a deferred final-
                multiply closure (runs ~6 steps later, once the broadcast has
                landed); the last window broadcasts via a K=1 PE matmul."""
                h, ihh = WINS[w]
                row = slice((h % 2) * 64, (h % 2) * 64 + 64)
                isl = slice(ihh * IHW, (ihh + 1) * IHW)
                ot = sbw.tile([64, IHW], F32, tag="ot", bufs=2, name="ot")
                zt = sbw.tile([1, IHW], F32, tag="zt", bufs=2, name="zt")
                rt = sbw.tile([1, IHW], F32, tag="rt", bufs=2, name="rt")
                for c in range(2):
                    cs = slice(c * 512, (c + 1) * 512)
                    nc.vector.tensor_copy(zt[:, cs], vc[c][64:65, :])
                    if fast:   # ACT is idle after the last EXP
                        nc.scalar.copy(ot[:, cs], vc[c][0:64, :])
                    else:
                        nc.vector.tensor_copy(ot[:, cs], vc[c][0:64, :])
                nc.vector.reciprocal_approx_fast(out=rt[:], in_=zt[:])
                if fast:
                    rtb = sbw.tile([1, IHW], BF16, tag="rtb", name="rtb")
                    nc.vector.tensor_copy(rtb[:], rt[:])
                    for c in range(2):
                        cs = slice(c * 512, (c + 1) * 512)
                        rbp = ps.tile([HD, 512], F32, tag=f"v{c}",
                                      name=f"ps_rb{c}")
                        nc.tensor.matmul(rbp[:], onesb[:], rtb[:, cs],
                                         start=True, stop=True)
                        nc.vector.tensor_mul(
                            ota[row, h // 2, ihh * IHW + c * 512:
                                ihh * IHW + (c + 1) * 512],
                            ot[:, cs], rbp[:])
                    return None
                rdram = dr.tile([1, IHW], F32, tag="rd")
                nc.sync.dma_start(rdram[:], rt[:])
                rb = sbw.tile([64, IHW], F32, tag="rb", bufs=2, name="rb")
                nc.sync.dma_start(rb[:], rdram[:].to_broadcast((64, IHW)))

                def finish():
                    nc.vector.tensor_mul(ota[row, h // 2, isl], ot[:], rb[:])
                return finish

            # ---------------- tail out-projection pieces ----------------
            st_parts = {}
            FULL, H2, H3 = slice(0, P), slice(0, 64), slice(64, P)

            def tail_mm(dst, o_rows, o, mt, c, start, stop):
                nc.tensor.matmul(
                    dst,
                    wo[o_rows, o, mt * P:(mt + 1) * P],
                    ota[o_rows, o, IHW + c * 512: IHW + (c + 1) * 512],
                    start=start, stop=stop,
                )

            def tail_prefill():
                for mt in range(3):
                    if mt < 2:
                        st = ps.tile([P, IHW], F32, tag=f"s{mt % 2}",
                                     name=f"ps_st{mt % 2}")
                        parts = (st,)
                    else:
                        parts = tuple(
                            ps.tile([P, 512], F32, tag=f"p{i}", name=f"ps_p{i}")
                            for i in range(2))
                    st_parts[mt] = parts
                    for c in range(2):
                        dst = (parts[c][:] if len(parts) == 2
                               else parts[0][:, c * 512:(c + 1) * 512])
                        tail_mm(dst, FULL, 0, mt, c, True, False)
                        tail_mm(dst, H2, 1, mt, c, False, False)
                # zero-accumulate keep-warm matmuls: hold PE p-state through
                # the normalize chain without changing the open groups.
                for i in range(18):
                    st0 = st_parts[0][0]
                    nc.tensor.matmul(st0[:, 0:512], zerot[:], warm[:],
                                     start=False, stop=False)

            # ---------------- the flat pipeline ----------------
            # warm-up: keep the PE busy through the input DMA so it reaches
            # full p-state before the lead-in fills (which are DMA-paced).
            for i in range(12):
                wps = ps.tile([64, 512], F32, tag=f"v{i % 2}",
                              name=f"ps_wm{i % 2}")
                nc.tensor.matmul(wps[:], warm[:, 0:64], warm[:],
                                 start=True, stop=True)

            # lead-in: K^T heads 0,1 j 0:512; Q^T heads 0,1 ih0 c0.
            # (Q0 c1 is emitted inside step 0, after the first half-EXP, so
            # the exp stream starts as soon as K0f0+Q0f0 land.)
            kq_fill(False, 0, 0, 0)
            kq_fill(True, 0, 0, 0)

            from collections import defaultdict
            pre = defaultdict(list)
            mid = defaultdict(list)

            def place(d, step, *fns):
                for i, f in enumerate(fns):
                    d[step + i].append(f)

            # w0: K0 quarters chunked+paced; V pair0 paced per jb
            place(pre, 1, *kq_chunks(False, 0, 0, 1))
            place(pre, 5, *kq_chunks(False, 0, 1, 0))
            place(pre, 9, *kq_chunks(False, 0, 1, 1))
            for it in range(NJB):
                place(mid, it, lambda it=it: v_fill(it, 0))
            # w1 (V-free): Q1 ih0 + first K1 quarter, 2-matmul pieces
            place(pre, 16, *kq_chunks(True, 1, 0, 0, nparts=4))
            place(pre, 20, *kq_chunks(True, 1, 0, 1, nparts=4))
            place(pre, 26, *kq_chunks(False, 1, 0, 0, nparts=4))
            # w2: V pair1 paced one step ahead of its PV; K1 quarters
            place(pre, 32, *kq_chunks(False, 1, 0, 1))
            place(pre, 36, *kq_chunks(False, 1, 1, 0))
            place(pre, 40, *kq_chunks(False, 1, 1, 1))
            for it in range(NJB):
                place(mid, 32 + it, lambda it=it: v_fill(it, 1))
            # w3 (V-free): Q0 ih1 (due w4), 2-matmul pieces
            place(pre, 48, *kq_chunks(True, 0, 1, 0, nparts=4))
            place(pre, 52, *kq_chunks(True, 0, 1, 1, nparts=4))
            # w4: Q1 ih1 (due w6); out-proj(ih0) row-tiles 0,1
            place(pre, 64, *kq_chunks(True, 1, 1, 0, nparts=4))
            place(pre, 68, *kq_chunks(True, 1, 1, 1, nparts=4))
            place(pre, 72, *oproj_chunks(0, 0))
            place(pre, 76, *oproj_chunks(1, 0))
            # w5: out-proj(ih0) row-tiles 2..5
            for i, mt in enumerate((2, 3, 4, 5)):
                place(pre, 80 + 4 * i, *oproj_chunks(mt, 0))
            # w6: out-proj(ih0) row-tiles 6,7; single batched store after
            place(pre, 96, *oproj_chunks(6, 0))
            place(pre, 100, *oproj_chunks(7, 0))
            place(pre, 103,
                  lambda: nc.sync.dma_start(out_t[:, :, 0:IHW], stg0[:]))

            es_q = {}
            vc = None
            NSTEP = NW * NJB
            for k in range(NSTEP + 2):
                for f in pre.get(k, ()):
                    f()
                if k < NSTEP:
                    w, jb = divmod(k, NJB)
                    h, ihh = WINS[w]
                    row = slice((h % 2) * 64, (h % 2) * 64 + 64)
                    ss = ps.tile([P, IHW], F32, tag=f"s{k % 2}",
                                 name=f"ps_s{k % 2}")
                    es = sbw.tile([P, IHW], BF16, tag="es", bufs=4, name="es")

                    def s_mm(c):
                        nc.tensor.matmul(
                            ss[:, c * 512:(c + 1) * 512],
                            ktp[row, h, jb * P:(jb + 1) * P],
                            qt[row, h // 2, ihh * IHW + c * 512:
                               ihh * IHW + (c + 1) * 512],
                            start=True, stop=True,
                        )
                    if k == 0:
                        # half-width exps: start the stream on Q0f0 alone
                        s_mm(0)
                        nc.scalar.activation(es[:, 0:512], ss[:, 0:512],
                                             EXP_FN)
                        kq_fill(True, 0, 0, 1)
                        s_mm(1)
                        nc.scalar.activation(es[:, 512:1024],
                                             ss[:, 512:1024], EXP_FN)
                    else:
                        s_mm(0)
                        s_mm(1)
                        nc.scalar.activation(es[:], ss[:], EXP_FN)
                    es_q[k] = es
                for f in mid.get(k, ()):
                    f()
                p = k - 2
                if p < 0:
                    continue
                w, pj = divmod(p, NJB)
                h, ihh = WINS[w]
                if pj == 0:
                    vc = [ps.tile([65, 512], F32, tag=f"v{c}", name=f"ps_v{c}")
                          for c in range(2)]
                pes = es_q.pop(p)
                for c in range(2):
                    nc.tensor.matmul(
                        vc[c][:],
                        vaug[:, pj, h, 0:65],
                        pes[:, c * 512:(c + 1) * 512],
                        start=(pj == 0), stop=(pj == NJB - 1),
                    )
                if pj == NJB - 1:
                    last = (w == NW - 1)
                    if last:
                        tail_prefill()
                    fin = norm_window(w, vc, fast=last)
                    if fin is not None:
                        place(pre, k + 6, fin)

            # tail: finish the ih1 out-projection; evacuation alternates
            # between the idle ACT and DVE engines.  Prefilled row-tiles
            # must finish first — they hold the s/p banks open.
            for mt in range(8):
                if mt in st_parts:
                    parts = st_parts[mt]
                else:
                    parts = (ps.tile([P, IHW], F32, tag=f"s{mt % 2}",
                                     name=f"ps_st{mt % 2}"),)
                for c in range(2):
                    cs = slice(c * 512, (c + 1) * 512)
                    dst = parts[c][:] if len(parts) == 2 else parts[0][:, cs]
                    if mt in st_parts:
                        tail_mm(dst, H3, 1, mt, c, False, True)
                    else:
                        tail_mm(dst, FULL, 0, mt, c, True, False)
                        tail_mm(dst, FULL, 1, mt, c, False, True)
                    if (2 * mt + c) % 2:
                        nc.scalar.copy(stg1[:, mt, cs], dst)
                    else:
                        nc.vector.tensor_copy(stg1[:, mt, cs], dst)
                nc.sync.dma_start(out_t[:, mt, IHW:NTOK], stg1[:, mt, :])

    nc.compile()
    return nc


_NC_CACHE = None


def _get_nc():
    global _NC_CACHE
    if _NC_CACHE is None:
        _NC_CACHE = build_nc()
    return _NC_CACHE


def _swz(a, po):
    """[po*128, rest] -> [128, po, rest] host swizzle (contiguous/partition)."""
    rest = a.shape[1]
    return np.ascontiguousarray(
        a.reshape(po, P, rest).transpose(1, 0, 2)).astype(NP_BF16)


def kernel(x, Wq, Wk, Wv, Wo, bo, _trace=False):
    x = np.asarray(x, dtype=np.float32)
    Wq = np.asarray(Wq, dtype=np.float32)
    Wk = np.asarray(Wk, dtype=np.float32)
    Wv = np.asarray(Wv, dtype=np.float32)
    Wo = np.asarray(Wo, dtype=np.float32)
    bo = np.asarray(bo, dtype=np.float32)
    B = x.shape[0]

    nc = _get_nc()
    in_maps = []
    for core in range(8):
        b, hg = divmod(core, 4)
        rows = slice(hg * DSH, (hg + 1) * DSH)
        xs = _swz(np.ascontiguousarray(x[b].T), CO)      # [128, 8, 2048]
        m = {"xq0a": np.ascontiguousarray(xs[:, 0:4, 0:512]),
             "xq0b": np.ascontiguousarray(xs[:, 4:8, 0:512])}
        for q in (1, 2, 3):
            m[f"xq{q}"] = np.ascontiguousarray(xs[:, :, q * 512:(q + 1) * 512])
        m["wqs"] = _swz(np.ascontiguousarray(Wq[rows, :].T), CO)
        m["wks"] = _swz(np.ascontiguousarray((Wk[rows, :] * SCALE).T), CO)
        m["wvs"] = _swz(np.ascontiguousarray(Wv[rows, :].T), CO)
        m["wos"] = _swz(np.ascontiguousarray(Wo[:, rows].T), 2)
        in_maps.append(m)

    res = bass_utils.run_bass_kernel_spmd(
        nc, in_maps, core_ids=list(range(8)), trace=_trace)

    out = np.zeros((B, NTOK, D), dtype=np.float32)
    for core in range(8):
        b = core // 4
        out[b] += res.results[core]["outt"].astype(np.float32).T
    out += bo
    if _trace:
        kernel.last_results = res
    return out
